# revision 17
# baseline (speedup 1.0000x reference)
"""Trainium2 Bass kernel for nn_MultiHeadAttention_9345848836102.

MHA: B=2, S=2048, D=1024, H=16 heads (DK=64), RoPE on Q/K, causal mask,
returns (out [B,S,D], attn [B,H,S,S]).

Sharding: 8 cores = 2 batches x 4 head-groups (4 heads per core).
Each core computes QKV projections for its 4 heads, RoPE, scores, softmax,
writes its attn slice, computes attn@V and a partial output projection
(contraction over its heads' slice of D); host sums the 4 partials per batch
and adds bo.

Device-side structure per core:
  - QT/KT computed head-dim-on-partitions ([128 = 2heads x 64, S]) directly
    from x^T via PE matmuls; V in natural [S, d] layout.
  - RoPE via partition pair-swap (stream_shuffle) + 2 muls + add on DVE.
  - Natural path: scores [q-part, k-free] -> exp (ACT, accum_out = rowsums)
    -> normalize (DVE per-partition scalar) -> DMA to attn.
  - Transposed path: scores^T [k-part, q-free] (separate matmuls) -> exp ->
    P^T used directly as matmul rhs for O^T = V^T @ P^T (no transposes).
  - O^T normalized during PSUM evacuation using rowsum reciprocals
    round-tripped through DRAM into [k-broadcast, q] layout.
  - Output projection: out_partial = O^T(stacked pair).T @ Wo^T rows.
"""

import os

os.environ.setdefault("BASS_NEVER_TRACE", "1")

import math

import numpy as np

import concourse.bass as bass
import concourse.tile as tile
from concourse import mybir
from concourse.bass_utils import run_bass_kernel_spmd

B, S, D, H = 2, 2048, 1024, 16
DK = D // H          # 64
HPC = 4              # heads per core
P = 128
NCORES = 8
THETA = 10000.0
NEG = -1.0e9
SCALE = 1.0 / math.sqrt(DK)   # 0.125

F32 = mybir.dt.float32
F32R = mybir.dt.float32r
AX = mybir.AxisListType
AF = mybir.ActivationFunctionType

NQT = S // P          # 16 query tiles of 128
NQB = S // 512        # 4 query blocks of 512
NDC = D // P          # 8 contraction chunks of 128

PAIRSWAP = [i ^ 1 for i in range(32)]

DBG = {}


def _wait_cap(inst):
    """Max sync-waits walrus codegen accepts on this instruction.

    The fused fp32/fp32r Matmult lowers to an LDWEIGHTS+MATMUL pair whose LW
    half has a single wait slot; most other instructions take 4.
    """
    return 1


def _split_waits(nc):
    """Move excess sync-waits onto preceding same-engine NoOps.

    Sequencers issue in order, so a wait satisfied on an earlier instruction
    of the same engine gates everything after it identically.
    """
    NOP_CAP = 1
    for f in nc.m.functions:
        for blk in f.blocks:
            insts = list(blk.instructions)
            out = []
            changed = False
            for inst in insts:
                si = getattr(inst, "sync_info", None)
                waits = list(si.on_wait) if si is not None and si.on_wait else []
                cap = _wait_cap(inst)
                if len(waits) > cap:
                    excess, keep = waits[:-cap], waits[-cap:]
                    while excess:
                        chunk, excess = excess[:NOP_CAP], excess[NOP_CAP:]
                        nop = mybir.InstNoOp(
                            name=f"I-wsplit{nc.next_id()}", ins=[], outs=[])
                        nop.engine = inst.engine
                        nop.sync_info = mybir.SyncInfo(on_wait=chunk, on_update=[])
                        out.append(nop)
                    inst.sync_info = mybir.SyncInfo(
                        on_wait=keep, on_update=list(si.on_update))
                    changed = True
                out.append(inst)
            if changed:
                blk.instructions = out


def build_bass(bench=False):
    nc = bass.Bass(trn_type="TRN2", target_bir_lowering=False, debug=False)

    # ---- I/O ----
    xT = nc.dram_tensor("xT", [D, S], F32R, kind="ExternalInput").ap()
    wqt = nc.dram_tensor("wqt", [D, HPC * DK], F32R, kind="ExternalInput").ap()
    wkt = nc.dram_tensor("wkt", [D, HPC * DK], F32R, kind="ExternalInput").ap()
    wvt = nc.dram_tensor("wvt", [D, HPC * DK], F32R, kind="ExternalInput").ap()
    wot = nc.dram_tensor("wot", [HPC * DK, D], F32R, kind="ExternalInput").ap()
    bqd = nc.dram_tensor("bqd", [HPC * DK], F32, kind="ExternalInput").ap()
    bkd = nc.dram_tensor("bkd", [HPC * DK], F32, kind="ExternalInput").ap()
    bvd = nc.dram_tensor("bvd", [HPC * DK], F32, kind="ExternalInput").ap()
    cosd = nc.dram_tensor("cosd", [P, S], F32, kind="ExternalInput").ap()
    sind = nc.dram_tensor("sind", [P, S], F32, kind="ExternalInput").ap()
    trin = nc.dram_tensor("trin", [P, P], F32, kind="ExternalInput").ap()
    trit = nc.dram_tensor("trit", [P, P], F32, kind="ExternalInput").ap()

    okind = "Internal" if bench else "ExternalOutput"
    attn = nc.dram_tensor("attn", [HPC, S, S], F32, kind=okind).ap()
    outp = nc.dram_tensor("outp", [S, D], F32, kind=okind).ap()
    tok = nc.dram_tensor("tok", [1, 1], F32, kind="ExternalOutput").ap() if bench else None
    if os.environ.get("KERNEL_DEBUG"):
        DBG["ot"] = nc.dram_tensor("dbg_ot", [HPC, DK, S], F32, kind="ExternalOutput").ap()
        DBG["pt"] = nc.dram_tensor("dbg_pt", [P, 2048], F32, kind="ExternalOutput").ap()

    with tile.TileContext(nc) as tc:
        _build(tc, xT, wqt, wkt, wvt, wot, bqd, bkd, bvd, cosd, sind,
               trin, trit, attn, outp, tok)
    _split_waits(nc)
    return nc


def _build(tc, xT, wqt, wkt, wvt, wot, bqd, bkd, bvd, cosd, sind,
           trin, trit, attn, outp, tok=None):
    from contextlib import ExitStack

    nc = tc.nc

    with ExitStack() as outer:
        const = outer.enter_context(tc.tile_pool(name="const", bufs=1))
        persist = outer.enter_context(tc.tile_pool(name="persist", bufs=1))
        dram = outer.enter_context(tc.tile_pool(name="dram", bufs=1, space="DRAM"))

        # ---- constants ----
        wq_sb = const.tile([P, NDC, HPC * DK], F32R, tag="wq")
        nc.sync.dma_start(wq_sb[:], wqt.rearrange("(dc p) m -> p dc m", p=P))
        wk_sb = const.tile([P, NDC, HPC * DK], F32R, tag="wk")
        nc.sync.dma_start(wk_sb[:], wkt.rearrange("(dc p) m -> p dc m", p=P))
        wv_sb = const.tile([P, NDC, HPC * DK], F32R, tag="wv")
        nc.sync.dma_start(wv_sb[:], wvt.rearrange("(dc p) m -> p dc m", p=P))
        wo_sb = const.tile([DK, HPC, D], F32R, tag="wo")
        nc.sync.dma_start(wo_sb[:], wot.rearrange("(h p) n -> p h n", p=DK))
        bq_sb = const.tile([P, 2], F32, tag="bq")
        nc.sync.dma_start(bq_sb[:], bqd.rearrange("(p2 p) -> p p2", p=P))
        bk_sb = const.tile([P, 2], F32, tag="bk")
        nc.sync.dma_start(bk_sb[:], bkd.rearrange("(p2 p) -> p p2", p=P))
        bv_sb = const.tile([P, HPC * DK], F32, tag="bv")
        nc.sync.dma_start(bv_sb[:], bvd.partition_broadcast(P))
        cos_sb = const.tile([P, S], F32, tag="cos")
        nc.sync.dma_start(cos_sb[:], cosd)
        sin_sb = const.tile([P, S], F32, tag="sin")
        nc.sync.dma_start(sin_sb[:], sind)
        trin_sb = const.tile([P, P], F32, tag="trin")
        nc.sync.dma_start(trin_sb[:], trin)
        trit_sb = const.tile([P, P], F32, tag="trit")
        nc.sync.dma_start(trit_sb[:], trit)
        zeros_sb = const.tile([P, S - P], F32, tag="zeros")
        nc.gpsimd.memset(zeros_sb[:], 0.0)

        # ---- persistent activations ----
        QT = [persist.tile([P, S], F32R, tag=f"qt{p2}", name=f"QT{p2}") for p2 in range(2)]
        KT = [persist.tile([P, S], F32R, tag=f"kt{p2}", name=f"KT{p2}") for p2 in range(2)]
        V = persist.tile([P, NQT, HPC * DK], F32R, tag="v")
        OT = [persist.tile([DK, S], F32R, tag=f"ot{h}", name=f"OT{h}")
              for h in range(HPC)]

        recd = dram.tile([HPC, S], F32, tag="recd")

        # ================= Phase 1: QKV projections =================
        with ExitStack() as ctx:
            xpool = ctx.enter_context(tc.tile_pool(name="xts", bufs=2))
            ppq = ctx.enter_context(tc.tile_pool(name="ppq", bufs=2, space="PSUM"))
            ppv = ctx.enter_context(tc.tile_pool(name="ppv", bufs=2, space="PSUM"))

            xT_r = xT.rearrange("(dc p) s -> p dc s", p=P)
            for sb in range(NQB):
                ssl = slice(sb * 512, (sb + 1) * 512)
                xts = xpool.tile([P, NDC, 512], F32R, tag="xts")
                nc.sync.dma_start(xts[:], xT_r[:, :, ssl])
                for p2 in range(2):
                    for w_sb, dst, b_sb in ((wq_sb, QT, bq_sb), (wk_sb, KT, bk_sb)):
                        ps = ppq.tile([P, 512], F32, tag="ppq")
                        for dc in range(NDC):
                            nc.tensor.matmul(
                                ps[:],
                                (w_sb[:, dc, p2 * P:(p2 + 1) * P]),
                                (xts[:, dc, :]),
                                start=(dc == 0), stop=(dc == NDC - 1),
                            )
                        nc.vector.tensor_scalar_add(
                            dst[p2][:, ssl], ps[:], b_sb[:, p2:p2 + 1])
                for sc in range(4):
                    ps = ppv.tile([P, HPC * DK], F32, tag="ppv")
                    for dc in range(NDC):
                        nc.tensor.matmul(
                            ps[:],
                            (xts[:, dc, sc * P:(sc + 1) * P]),
                            (wv_sb[:, dc, :]),
                            start=(dc == 0), stop=(dc == NDC - 1),
                        )
                    nc.vector.tensor_add(V[:, sb * 4 + sc, :], ps[:], bv_sb[:])

        # ================= Phase 2: RoPE on QT/KT =================
        with ExitStack() as ctx:
            rpool = ctx.enter_context(tc.tile_pool(name="rope", bufs=2))
            for t in (QT[0], QT[1], KT[0], KT[1]):
                sw = rpool.tile([P, S], F32, tag="sw")
                nc.vector.stream_shuffle(sw[:], t[:], PAIRSWAP)
                nc.vector.tensor_mul(sw[:], sw[:], sin_sb[:])
                tm = rpool.tile([P, S], F32, tag="tm")
                nc.vector.tensor_mul(tm[:], t[:], cos_sb[:])
                nc.vector.tensor_add(t[:], tm[:], sw[:])

        # ================= Phase 3: attention per head =================
        with ExitStack() as ctx:
            stripe_p = ctx.enter_context(tc.tile_pool(name="stripe", bufs=2))
            pt_p = ctx.enter_context(tc.tile_pool(name="pt", bufs=4))
            small_p = ctx.enter_context(tc.tile_pool(name="small", bufs=2))
            recb_p = ctx.enter_context(tc.tile_pool(name="recb", bufs=2))
            psA = ctx.enter_context(tc.tile_pool(name="psA", bufs=2, space="PSUM"))
            psQ = ctx.enter_context(tc.tile_pool(name="psQ", bufs=2, space="PSUM"))
            psO = ctx.enter_context(tc.tile_pool(name="psO", bufs=2, space="PSUM"))

            def nat_stripe(h, qt, rec_sb):
                """Natural-layout scores stripe: attn row-block + softmax stats."""
                p2, hh = h // 2, h % 2
                hsl = slice(hh * DK, (hh + 1) * DK)
                qth, kth = QT[p2][hsl, :], KT[p2][hsl, :]
                w = (qt + 1) * P
                ntile = (w + 511) // 512
                stripe = stripe_p.tile([P, S], F32, tag="stripe", name="stripe")
                sums = small_p.tile([P, 4], F32, tag="sums", name="sums")
                for c in range(ntile):
                    cw = min(512, w - c * 512)
                    ps = psA.tile([P, 512], F32, tag="psA", name="psA_t")
                    nc.tensor.matmul(
                        ps[:, :cw],
                        (qth[:, qt * P:(qt + 1) * P]),
                        (kth[:, c * 512:c * 512 + cw]),
                        start=True, stop=True,
                    )
                    if c == ntile - 1:
                        off = qt * P - c * 512
                        nc.vector.tensor_add(
                            ps[:, off:off + P], ps[:, off:off + P], trin_sb[:])
                    nc.scalar.activation(
                        stripe[:, c * 512:c * 512 + cw], ps[:, :cw],
                        AF.Exp, scale=SCALE,
                        accum_out=sums[:, c:c + 1],
                    )
                rtmp = small_p.tile([P, 1], F32, tag="rtmp", name="rtmp")
                if ntile > 1:
                    nc.vector.reduce_sum(rtmp[:], sums[:, :ntile], axis=AX.X)
                    nc.vector.reciprocal(rec_sb[:, qt:qt + 1], rtmp[:])
                else:
                    nc.vector.reciprocal(rec_sb[:, qt:qt + 1], sums[:, 0:1])
                nc.vector.tensor_scalar_mul(
                    stripe[:, :w], stripe[:, :w], rec_sb[:, qt:qt + 1])
                rows = slice(qt * P, (qt + 1) * P)
                nc.sync.dma_start(attn[h, rows, 0:w], stripe[:, :w])
                if w < S:
                    nc.sync.dma_start(attn[h, rows, w:S], zeros_sb[:, :S - w])
                if qt == NQT - 1:
                    # reciprocals -> DRAM, re-read later in [k, q] layout
                    nc.sync.dma_start(
                        recd[h].rearrange("(t p) -> p t", p=P), rec_sb[:])

            def st_quad(h, qb, g, ot_ps):
                """Transposed-scores quad (4 k-tiles): S^T -> exp -> O^T accum."""
                p2, hh = h // 2, h % 2
                hsl = slice(hh * DK, (hh + 1) * DK)
                qth, kth = QT[p2][hsl, :], KT[p2][hsl, :]
                band = (g == qb)
                stq = psQ.tile([P, 1024], F32, tag="psQ", name="stq")
                ptq = pt_p.tile([P, 1024], F32R, tag="ptq", name="ptq")
                stq2 = psQ.tile([P, 1024], F32, tag="psQ", name="stq2")
                ptq2 = pt_p.tile([P, 1024], F32R, tag="ptq", name="ptq2")
                for i in range(4):
                    kt = g * 4 + i
                    r = 128 * i if band else 0
                    sq, pq = (stq, ptq) if i < 2 else (stq2, ptq2)
                    j = i % 2
                    nc.tensor.matmul(
                        sq[:, j * 512 + r:(j + 1) * 512],
                        (kth[:, kt * P:(kt + 1) * P]),
                        (qth[:, qb * 512 + r:(qb + 1) * 512]),
                        start=True, stop=True,
                    )
                    if band:
                        nc.vector.tensor_add(
                            sq[:, j * 512 + r:j * 512 + r + P],
                            sq[:, j * 512 + r:j * 512 + r + P],
                            trit_sb[:])
                        nc.scalar.activation(
                            pq[:, j * 512 + r:(j + 1) * 512],
                            sq[:, j * 512 + r:(j + 1) * 512],
                            AF.Exp, scale=SCALE)
                if not band:
                    nc.scalar.activation(ptq[:], stq[:], AF.Exp, scale=SCALE)
                    nc.scalar.activation(ptq2[:], stq2[:], AF.Exp, scale=SCALE)
                if DBG and h == 0 and qb == 0:
                    nc.sync.dma_start(DBG["pt"][:, 0:1024].bitcast(F32R), ptq[:])
                    nc.sync.dma_start(DBG["pt"][:, 1024:2048].bitcast(F32R), ptq2[:])
                for i in range(4):
                    kt = g * 4 + i
                    # masked q < kt*128 region of band tiles is never written
                    # nor read: matmul consumes only the valid sub-slice (the
                    # first matmul in the psum group is always full-width, so
                    # has_written covers the whole bank)
                    r = 128 * i if band else 0
                    pq = ptq if i < 2 else ptq2
                    j = i % 2
                    nc.tensor.matmul(
                        ot_ps[:, r:512],
                        (V[:, kt, h * DK:(h + 1) * DK]),
                        (pq[:, j * 512 + r:(j + 1) * 512]),
                        start=(kt == 0), stop=(kt == 4 * qb + 3),
                        skip_group_check=True,
                    )

            def st_evac(h, qb, ot_ps):
                qsl = slice(qb * 512, (qb + 1) * 512)
                recb = recb_p.tile([DK, 512], F32, tag="recb", name="recb")
                nc.sync.dma_start(recb[:], recd[h, qsl].partition_broadcast(DK))
                nc.vector.tensor_mul(OT[h][:, qsl], ot_ps[:], recb[:])

            # Interleave natural path of head h with transposed path of head
            # h-1 so PE never waits long on ACT exp (in-order engine streams).
            def st_tasks_for(h):
                if h < 0:
                    return
                for qb in range(NQB):
                    ot_ps = psO.tile([DK, 512], F32, tag="psO", name="ot_ps")
                    for g in range(qb + 1):
                        yield (st_quad, (h, qb, g, ot_ps))
                    yield (st_evac, (h, qb, ot_ps))

            def nat_tasks_for(h):
                if h >= HPC:
                    return
                rec_sb = small_p.tile([P, NQT], F32, tag="rec", name="rec_sb")
                for qt in range(NQT):
                    yield (nat_stripe, (h, qt, rec_sb))

            for h in range(HPC + 1):
                nat = list(nat_tasks_for(h))
                st = list(st_tasks_for(h - 1))
                n, m = len(nat), len(st)
                ni = si = 0
                for step in range(n + m):
                    # round-robin proportionally
                    if ni * max(m, 1) <= si * max(n, 1) and ni < n:
                        fn, args = nat[ni]; ni += 1
                    elif si < m:
                        fn, args = st[si]; si += 1
                    else:
                        fn, args = nat[ni]; ni += 1
                    fn(*args)

        if DBG:
            for h in range(HPC):
                nc.sync.dma_start(DBG["ot"][h].bitcast(F32R), OT[h][:])

        # ================= Phase 4: output projection =================
        with ExitStack() as ctx:
            psF = ctx.enter_context(tc.tile_pool(name="psF", bufs=2, space="PSUM"))
            osb = ctx.enter_context(tc.tile_pool(name="osb", bufs=3))
            for qt in range(NQT):
                for nb in range(2):
                    ps = psF.tile([P, 512], F32, tag="psF")
                    for h in range(HPC):
                        nc.tensor.matmul(
                            ps[:],
                            (OT[h][:, qt * P:(qt + 1) * P]),
                            (wo_sb[:, h, nb * 512:(nb + 1) * 512]),
                            start=(h == 0), stop=(h == HPC - 1),
                        )
                    ob = osb.tile([P, 512], F32, tag="ob")
                    nc.vector.tensor_copy(ob[:], ps[:])
                    nc.sync.dma_start(
                        outp[qt * P:(qt + 1) * P, nb * 512:(nb + 1) * 512], ob[:])
            if tok is not None:
                nc.sync.dma_start(tok, zeros_sb[0:1, 0:1])


_NC_CACHE = None


def _get_nc():
    global _NC_CACHE
    if _NC_CACHE is None:
        _NC_CACHE = build_bass()
    return _NC_CACHE


def _rope_tables(token_positions):
    half = DK // 2
    inv_freq = THETA ** (-np.arange(half, dtype=np.float64) * 2.0 / DK)
    ang = token_positions.astype(np.float64)[:, None] * inv_freq[None, :]  # [S, half]
    cos = np.cos(ang).astype(np.float32)   # [S, 32]
    sin = np.sin(ang).astype(np.float32)
    cosT = np.repeat(cos.T, 2, axis=0)     # [64, S]
    sinT = np.repeat(sin.T, 2, axis=0)
    sign = np.where(np.arange(DK) % 2 == 0, -1.0, 1.0).astype(np.float32)
    sinS = sinT * sign[:, None]
    cos128 = np.tile(cosT, (2, 1))         # [128, S]
    sin128 = np.tile(sinS, (2, 1))
    return np.ascontiguousarray(cos128), np.ascontiguousarray(sin128)


def make_in_maps(inputs):
    x = np.asarray(inputs["x"], dtype=np.float32)
    cos128, sin128 = _rope_tables(np.asarray(inputs["token_positions"]))
    idx = np.arange(P)
    trin_np = np.where(idx[None, :] <= idx[:, None], 0.0, NEG).astype(np.float32)
    trit_np = np.where(idx[None, :] < idx[:, None], NEG, 0.0).astype(np.float32)

    WqT = np.ascontiguousarray(np.asarray(inputs["Wq"], np.float32).T)
    WkT = np.ascontiguousarray(np.asarray(inputs["Wk"], np.float32).T)
    WvT = np.ascontiguousarray(np.asarray(inputs["Wv"], np.float32).T)
    WoT = np.ascontiguousarray(np.asarray(inputs["Wo"], np.float32).T)
    bq = np.asarray(inputs["bq"], np.float32)
    bk = np.asarray(inputs["bk"], np.float32)
    bv = np.asarray(inputs["bv"], np.float32)

    xTs = [np.ascontiguousarray(x[b].T) for b in range(B)]

    in_maps = []
    for c in range(NCORES):
        b = c // (NCORES // B)
        hg = c % (NCORES // B)
        lo, hi = hg * HPC * DK, (hg + 1) * HPC * DK
        in_maps.append({
            "xT": xTs[b],
            "wqt": np.ascontiguousarray(WqT[:, lo:hi]),
            "wkt": np.ascontiguousarray(WkT[:, lo:hi]),
            "wvt": np.ascontiguousarray(WvT[:, lo:hi]),
            "wot": np.ascontiguousarray(WoT[lo:hi, :]),
            "bqd": np.ascontiguousarray(bq[lo:hi]),
            "bkd": np.ascontiguousarray(bk[lo:hi]),
            "bvd": np.ascontiguousarray(bv[lo:hi]),
            "cosd": cos128,
            "sind": sin128,
            "trin": trin_np,
            "trit": trit_np,
        })
    return in_maps


def kernel(x, Wq, bq, Wk, bk, Wv, bv, Wo, bo, token_positions, mask):
    nc = _get_nc()
    in_maps = make_in_maps({
        "x": x, "Wq": Wq, "Wk": Wk, "Wv": Wv, "Wo": Wo,
        "bq": bq, "bk": bk, "bv": bv,
        "token_positions": token_positions,
    })

    res = run_bass_kernel_spmd(nc, in_maps, core_ids=list(range(NCORES)))
    results = res.results

    attn_full = np.empty((B, H, S, S), dtype=np.float32)
    out_full = np.empty((B, S, D), dtype=np.float32)
    bo = np.asarray(bo, np.float32)
    for b in range(B):
        acc = None
        for hg in range(NCORES // B):
            c = b * (NCORES // B) + hg
            attn_full[b, hg * HPC:(hg + 1) * HPC] = results[c]["attn"]
            acc = results[c]["outp"] if acc is None else acc + results[c]["outp"]
        out_full[b] = acc + bo[None, :]
    return out_full, attn_full


# revision 31
# speedup vs baseline: 2.1565x; 2.1565x over previous
"""Trainium2 Bass kernel for nn_MultiHeadAttention_9345848836102.

MHA: B=2, S=2048, D=1024, H=16 heads (DK=64), RoPE on Q/K, causal mask,
returns (out [B,S,D], attn [B,H,S,S]).

Sharding: 8 cores = 2 batches x 4 head-groups (4 heads per core).
Each core computes QKV projections for its 4 heads, RoPE, scores, softmax,
writes its attn slice, computes attn@V and a partial output projection
(contraction over its heads' slice of D); host sums the 4 partials per batch
and adds bo.

Device-side structure per core:
  - QT/KT computed head-dim-on-partitions ([128 = 2heads x 64, S]) directly
    from x^T via PE matmuls; V in natural [S, d] layout.
  - RoPE via partition pair-swap (stream_shuffle) + 2 muls + add on DVE.
  - Natural path: scores [q-part, k-free] -> exp (ACT, accum_out = rowsums)
    -> normalize (DVE per-partition scalar) -> DMA to attn.
  - Transposed path: scores^T [k-part, q-free] (separate matmuls) -> exp ->
    P^T used directly as matmul rhs for O^T = V^T @ P^T (no transposes).
  - O^T normalized during PSUM evacuation using rowsum reciprocals
    round-tripped through DRAM into [k-broadcast, q] layout.
  - Output projection: out_partial = O^T(stacked pair).T @ Wo^T rows.
"""

import os

os.environ.setdefault("BASS_NEVER_TRACE", "1")

import math

import numpy as np

import concourse.bass as bass
import concourse.tile as tile
from concourse import mybir
from concourse.bass_utils import run_bass_kernel_spmd

B, S, D, H = 2, 2048, 1024, 16
DK = D // H          # 64
HPC = 4              # heads per core
P = 128
NCORES = 8
THETA = 10000.0
NEG = -1.0e9
SCALE = 1.0 / math.sqrt(DK)   # 0.125

F32 = mybir.dt.float32
F32R = mybir.dt.float32r
AX = mybir.AxisListType
AF = mybir.ActivationFunctionType

NQT = S // P          # 16 query tiles of 128
NQB = S // 512        # 4 query blocks of 512
NDC = D // P          # 8 contraction chunks of 128

PAIRSWAP = [i ^ 1 for i in range(32)]

DBG = {}


def _wait_cap(inst):
    """Max sync-waits walrus codegen accepts on this instruction.

    The fused fp32/fp32r Matmult lowers to an LDWEIGHTS+MATMUL pair whose LW
    half has a single wait slot; most other instructions take 4.
    """
    return 1


def _split_waits(nc):
    """Move excess sync-waits onto preceding same-engine NoOps.

    Sequencers issue in order, so a wait satisfied on an earlier instruction
    of the same engine gates everything after it identically.
    """
    NOP_CAP = 1
    for f in nc.m.functions:
        for blk in f.blocks:
            insts = list(blk.instructions)
            out = []
            changed = False
            for inst in insts:
                si = getattr(inst, "sync_info", None)
                waits = list(si.on_wait) if si is not None and si.on_wait else []
                cap = _wait_cap(inst)
                if len(waits) > cap:
                    excess, keep = waits[:-cap], waits[-cap:]
                    while excess:
                        chunk, excess = excess[:NOP_CAP], excess[NOP_CAP:]
                        nop = mybir.InstNoOp(
                            name=f"I-wsplit{nc.next_id()}", ins=[], outs=[])
                        nop.engine = inst.engine
                        nop.sync_info = mybir.SyncInfo(on_wait=chunk, on_update=[])
                        out.append(nop)
                    inst.sync_info = mybir.SyncInfo(
                        on_wait=keep, on_update=list(si.on_update))
                    changed = True
                out.append(inst)
            if changed:
                blk.instructions = out


def build_bass(bench=False, reps=1):
    nc = bass.Bass(trn_type="TRN2", target_bir_lowering=False, debug=False)

    # ---- I/O ----
    xT = nc.dram_tensor("xT", [D, S], F32R, kind="ExternalInput").ap()
    wqt = nc.dram_tensor("wqt", [D, HPC * DK], F32R, kind="ExternalInput").ap()
    wkt = nc.dram_tensor("wkt", [D, HPC * DK], F32R, kind="ExternalInput").ap()
    wvt = nc.dram_tensor("wvt", [D, HPC * DK], F32R, kind="ExternalInput").ap()
    wot = nc.dram_tensor("wot", [HPC * DK, D], F32R, kind="ExternalInput").ap()
    bqd = nc.dram_tensor("bqd", [HPC * DK], F32, kind="ExternalInput").ap()
    bkd = nc.dram_tensor("bkd", [HPC * DK], F32, kind="ExternalInput").ap()
    bvd = nc.dram_tensor("bvd", [HPC * DK], F32, kind="ExternalInput").ap()
    cosd = nc.dram_tensor("cosd", [P, S], F32, kind="ExternalInput").ap()
    sind = nc.dram_tensor("sind", [P, S], F32, kind="ExternalInput").ap()
    trin = nc.dram_tensor("trin", [P, P], F32, kind="ExternalInput").ap()
    trit = nc.dram_tensor("trit", [P, P], F32, kind="ExternalInput").ap()

    okind = "Internal" if bench else "ExternalOutput"
    attn = nc.dram_tensor("attn", [HPC, S, S], F32, kind=okind).ap()
    outp = nc.dram_tensor("outp", [S, D], F32, kind=okind).ap()
    tok = nc.dram_tensor("tok", [1, 1], F32, kind="ExternalOutput").ap() if bench else None
    extra = [(nc.dram_tensor(f"attn_r{r}", [HPC, S, S], F32, kind="Internal").ap(),
              nc.dram_tensor(f"outp_r{r}", [S, D], F32, kind="Internal").ap())
             for r in range(1, reps)]
    if os.environ.get("KERNEL_DEBUG"):
        DBG["ot"] = nc.dram_tensor("dbg_ot", [HPC, DK, S], F32, kind="ExternalOutput").ap()
        DBG["pt"] = nc.dram_tensor("dbg_pt", [P, 2048], F32, kind="ExternalOutput").ap()

    with tile.TileContext(nc) as tc:
        for r in range(reps):
            a_, o_ = (attn, outp) if r == 0 else extra[r - 1]
            _build(tc, xT, wqt, wkt, wvt, wot, bqd, bkd, bvd, cosd, sind,
                   trin, trit, a_, o_, tok if r == reps - 1 else None)
    _split_waits(nc)
    return nc


def _build(tc, xT, wqt, wkt, wvt, wot, bqd, bkd, bvd, cosd, sind,
           trin, trit, attn, outp, tok=None):
    from contextlib import ExitStack

    nc = tc.nc

    with ExitStack() as outer:
        const = outer.enter_context(tc.tile_pool(name="const", bufs=1))
        persist = outer.enter_context(tc.tile_pool(name="persist", bufs=1))
        dram = outer.enter_context(tc.tile_pool(name="dram", bufs=1, space="DRAM"))

        # ---- constants ----
        wq_sb = const.tile([P, NDC, HPC * DK], F32R, tag="wq")
        nc.sync.dma_start(wq_sb[:], wqt.rearrange("(dc p) m -> p dc m", p=P))
        wk_sb = const.tile([P, NDC, HPC * DK], F32R, tag="wk")
        nc.sync.dma_start(wk_sb[:], wkt.rearrange("(dc p) m -> p dc m", p=P))
        wv_sb = const.tile([P, NDC, HPC * DK], F32R, tag="wv")
        nc.sync.dma_start(wv_sb[:], wvt.rearrange("(dc p) m -> p dc m", p=P))
        wo_sb = const.tile([DK, HPC, D], F32R, tag="wo")
        nc.sync.dma_start(wo_sb[:], wot.rearrange("(h p) n -> p h n", p=DK))
        bq_sb = const.tile([P, 2], F32, tag="bq")
        nc.sync.dma_start(bq_sb[:], bqd.rearrange("(p2 p) -> p p2", p=P))
        bk_sb = const.tile([P, 2], F32, tag="bk")
        nc.sync.dma_start(bk_sb[:], bkd.rearrange("(p2 p) -> p p2", p=P))
        bv_sb = const.tile([P, HPC * DK], F32, tag="bv")
        nc.sync.dma_start(bv_sb[:], bvd.partition_broadcast(P))
        cos_sb = const.tile([P, S], F32, tag="cos")
        nc.sync.dma_start(cos_sb[:], cosd)
        sin_sb = const.tile([P, S], F32, tag="sin")
        nc.sync.dma_start(sin_sb[:], sind)
        trin_sb = const.tile([P, P], F32, tag="trin")
        nc.sync.dma_start(trin_sb[:], trin)
        trit_sb = const.tile([P, P], F32, tag="trit")
        nc.sync.dma_start(trit_sb[:], trit)
        zeros_sb = const.tile([P, 512], F32, tag="zeros")
        nc.gpsimd.memset(zeros_sb[:], 0.0)

        # ---- persistent activations ----
        QT = [persist.tile([P, S], F32R, tag=f"qt{p2}", name=f"QT{p2}") for p2 in range(2)]
        KT = [persist.tile([P, S], F32R, tag=f"kt{p2}", name=f"KT{p2}") for p2 in range(2)]
        V = persist.tile([P, NQT, HPC * DK], F32R, tag="v")
        OT = [persist.tile([DK, S], F32R, tag=f"ot{h}", name=f"OT{h}")
              for h in range(HPC)]

        recd = dram.tile([HPC, S], F32, tag="recd")
        stripe_p = outer.enter_context(tc.tile_pool(name="stripe", bufs=3))

        xT_r = xT.rearrange("(dc p) s -> p dc s", p=P)

        # ================= Phase 1: Q/K projections (V deferred) ==========
        with ExitStack() as ctx:
            xpool = ctx.enter_context(tc.tile_pool(name="xts", bufs=2))

            def rope(t):
                sw = stripe_p.tile([P, S], F32, tag="stripe", name="sw")
                nc.vector.stream_shuffle(sw[:], t[:], PAIRSWAP)
                nc.vector.tensor_mul(sw[:], sw[:], sin_sb[:])
                tm = stripe_p.tile([P, S], F32, tag="stripe", name="tm")
                nc.vector.tensor_mul(tm[:], t[:], cos_sb[:])
                nc.vector.tensor_add(t[:], tm[:], sw[:])

            with tc.tile_pool(name="ppq", bufs=2, space="PSUM") as ppq:
                for sb in range(NQB):
                    ssl = slice(sb * 512, (sb + 1) * 512)
                    xts = xpool.tile([P, NDC, 512], F32R, tag="xts")
                    nc.sync.dma_start(xts[:], xT_r[:, :, ssl])
                    for p2 in range(2):
                        for w_sb, dst, b_sb in ((wq_sb, QT, bq_sb),
                                                (wk_sb, KT, bk_sb)):
                            ps = ppq.tile([P, 512], F32, tag="ppq")
                            for dc in range(NDC):
                                nc.tensor.matmul(
                                    ps[:],
                                    (w_sb[:, dc, p2 * P:(p2 + 1) * P]),
                                    (xts[:, dc, :]),
                                    start=(dc == 0), stop=(dc == NDC - 1),
                                )
                            nc.vector.tensor_scalar_add(
                                dst[p2][:, ssl], ps[:], b_sb[:, p2:p2 + 1])
            # pair-0 rope now; pair-1 rope rides in the h=0 task stream
            rope(QT[0])
            rope(KT[0])

        # ================= Phase 3: attention per head =================
        with ExitStack() as ctx:
            xpool = ctx.enter_context(tc.tile_pool(name="xts2", bufs=1))
            pt_p = ctx.enter_context(tc.tile_pool(name="pt", bufs=4))
            small_p = ctx.enter_context(tc.tile_pool(name="small", bufs=2))
            recb_p = ctx.enter_context(tc.tile_pool(name="recb", bufs=2))
            psW = ctx.enter_context(tc.tile_pool(name="psW", bufs=3, space="PSUM"))
            psO = ctx.enter_context(tc.tile_pool(name="psO", bufs=1, space="PSUM"))
            ppv = ctx.enter_context(tc.tile_pool(name="ppv", bufs=1, space="PSUM"))

            def v_proj(sb):
                ssl = slice(sb * 512, (sb + 1) * 512)
                xts = xpool.tile([P, NDC, 512], F32R, tag="xts2", name="xts")
                nc.sync.dma_start(xts[:], xT_r[:, :, ssl])
                for sc in range(4):
                    ps = ppv.tile([P, HPC * DK], F32, tag="ppv", name="ps_v")
                    for dc in range(NDC):
                        nc.tensor.matmul(
                            ps[:],
                            (xts[:, dc, sc * P:(sc + 1) * P]),
                            (wv_sb[:, dc, :]),
                            start=(dc == 0), stop=(dc == NDC - 1),
                        )
                    nc.vector.tensor_add(V[:, sb * 4 + sc, :], ps[:], bv_sb[:])

            def nat_stripe(h, qt, rec_sb):
                """Natural-layout scores stripe: attn row-block + softmax stats."""
                p2, hh = h // 2, h % 2
                hsl = slice(hh * DK, (hh + 1) * DK)
                qth, kth = QT[p2][hsl, :], KT[p2][hsl, :]
                w = (qt + 1) * P
                ntile = (w + 1023) // 1024
                stripe = stripe_p.tile([P, S], F32, tag="stripe", name="stripe")
                sums = small_p.tile([P, 2], F32, tag="sums", name="sums")
                for ti in range(ntile):
                    tw = min(1024, w - ti * 1024)
                    ps = psW.tile([P, 1024], F32, tag="psW", name="psA_t")
                    for c2 in range((tw + 511) // 512):
                        cw = min(512, tw - c2 * 512)
                        nc.tensor.matmul(
                            ps[:, c2 * 512:c2 * 512 + cw],
                            (qth[:, qt * P:(qt + 1) * P]),
                            (kth[:, ti * 1024 + c2 * 512:
                                 ti * 1024 + c2 * 512 + cw]),
                            start=True, stop=True,
                        )
                    if ti == ntile - 1:
                        off = qt * P - ti * 1024
                        nc.vector.tensor_add(
                            ps[:, off:off + P], ps[:, off:off + P], trin_sb[:])
                    nc.scalar.activation(
                        stripe[:, ti * 1024:ti * 1024 + tw], ps[:, :tw],
                        AF.Exp, scale=SCALE,
                        accum_out=sums[:, ti:ti + 1],
                    )
                rtmp = small_p.tile([P, 1], F32, tag="rtmp", name="rtmp")
                if ntile > 1:
                    nc.vector.reduce_sum(rtmp[:], sums[:, :ntile], axis=AX.X)
                    nc.vector.reciprocal(rec_sb[:, qt:qt + 1], rtmp[:])
                else:
                    nc.vector.reciprocal(rec_sb[:, qt:qt + 1], sums[:, 0:1])
                nc.vector.tensor_scalar_mul(
                    stripe[:, :w], stripe[:, :w], rec_sb[:, qt:qt + 1])
                rows = slice(qt * P, (qt + 1) * P)
                nc.sync.dma_start(attn[h, rows, 0:w], stripe[:, :w])
                z = w
                while z < S:
                    zw = min(512, S - z)
                    nc.sync.dma_start(attn[h, rows, z:z + zw], zeros_sb[:, :zw])
                    z += zw
                if qt % 4 == 3:
                    # reciprocals -> DRAM per q-block, re-read in [k, q] layout
                    qb = qt // 4
                    nc.sync.dma_start(
                        recd[h, qb * 512:(qb + 1) * 512].rearrange(
                            "(t p) -> p t", p=P),
                        rec_sb[:, qb * 4:qb * 4 + 4])

            def st_quad(h, qb, g, ot_ps):
                """Transposed-scores quad (4 k-tiles): S^T -> exp -> O^T accum."""
                p2, hh = h // 2, h % 2
                hsl = slice(hh * DK, (hh + 1) * DK)
                qth, kth = QT[p2][hsl, :], KT[p2][hsl, :]
                band = (g == qb)
                stq = psW.tile([P, 1024], F32, tag="psW", name="stq")
                ptq = pt_p.tile([P, 1024], F32R, tag="ptq", name="ptq")
                stq2 = psW.tile([P, 1024], F32, tag="psW", name="stq2")
                ptq2 = pt_p.tile([P, 1024], F32R, tag="ptq", name="ptq2")
                for i in range(4):
                    kt = g * 4 + i
                    r = 128 * i if band else 0
                    sq, pq = (stq, ptq) if i < 2 else (stq2, ptq2)
                    j = i % 2
                    nc.tensor.matmul(
                        sq[:, j * 512 + r:(j + 1) * 512],
                        (kth[:, kt * P:(kt + 1) * P]),
                        (qth[:, qb * 512 + r:(qb + 1) * 512]),
                        start=True, stop=True,
                    )
                    if band:
                        nc.vector.tensor_add(
                            sq[:, j * 512 + r:j * 512 + r + P],
                            sq[:, j * 512 + r:j * 512 + r + P],
                            trit_sb[:])
                        nc.scalar.activation(
                            pq[:, j * 512 + r:(j + 1) * 512],
                            sq[:, j * 512 + r:(j + 1) * 512],
                            AF.Exp, scale=SCALE)
                if not band:
                    nc.scalar.activation(ptq[:], stq[:], AF.Exp, scale=SCALE)
                    nc.scalar.activation(ptq2[:], stq2[:], AF.Exp, scale=SCALE)
                if DBG and h == 0 and qb == 0:
                    nc.sync.dma_start(DBG["pt"][:, 0:1024].bitcast(F32R), ptq[:])
                    nc.sync.dma_start(DBG["pt"][:, 1024:2048].bitcast(F32R), ptq2[:])
                for i in range(4):
                    kt = g * 4 + i
                    # masked q < kt*128 region of band tiles is never written
                    # nor read: matmul consumes only the valid sub-slice (the
                    # first matmul in the psum group is always full-width, so
                    # has_written covers the whole bank)
                    r = 128 * i if band else 0
                    pq = ptq if i < 2 else ptq2
                    j = i % 2
                    nc.tensor.matmul(
                        ot_ps[:, r:512],
                        (V[:, kt, h * DK:(h + 1) * DK]),
                        (pq[:, j * 512 + r:(j + 1) * 512]),
                        start=(kt == 0), stop=(kt == 4 * qb + 3),
                        skip_group_check=True,
                    )

            def recb_fetch(h, qb, recb):
                qsl = slice(qb * 512, (qb + 1) * 512)
                nc.sync.dma_start(recb[:], recd[h, qsl].partition_broadcast(DK))

            def st_evac(h, qb, ot_ps, recb):
                qsl = slice(qb * 512, (qb + 1) * 512)
                nc.vector.tensor_mul(OT[h][:, qsl], ot_ps[:], recb[:])

            def outproj_block(qt, nb):
                ps = psW.tile([P, 1024], F32, tag="psW",
                              name="ps_o")[:, :512]
                for hx in range(HPC):
                    nc.tensor.matmul(
                        ps[:],
                        (OT[hx][:, qt * P:(qt + 1) * P]),
                        (wo_sb[:, hx, nb * 512:(nb + 1) * 512]),
                        start=(hx == 0), stop=(hx == HPC - 1),
                    )
                ob = stripe_p.tile([P, 2048], F32, tag="stripe",
                                   name="ob")[:, :512]
                nc.vector.tensor_copy(ob[:], ps[:])
                nc.sync.dma_start(
                    outp[qt * P:(qt + 1) * P, nb * 512:(nb + 1) * 512], ob[:])

            # Both score paths of the same head run together, per q-block:
            # the natural stripes of block qb write that block's reciprocals
            # to DRAM, then the transposed quads + O^T evac of qb follow,
            # interleaved so no engine waits long on another (in-order
            # engine streams).  V-projection and pair-1 rope ride along with
            # head 0; the output projection rides with head 3's evacs.
            def interleave(a, b):
                n, m = len(a), len(b)
                ai = bi = 0
                out = []
                for _ in range(n + m):
                    if bi * max(n, 1) <= ai * max(m, 1) and bi < m:
                        out.append(b[bi]); bi += 1
                    elif ai < n:
                        out.append(a[ai]); ai += 1
                    else:
                        out.append(b[bi]); bi += 1
                return out

            def st_slot(h, qb):
                """All transposed-path work for one (head, q-block)."""
                ot_ps = psO.tile([DK, 512], F32, tag="psO", name="ot_ps")
                recb = recb_p.tile([DK, 512], F32, tag="recb", name="recb")
                out = [(recb_fetch, (h, qb, recb))]
                out += [(st_quad, (h, qb, g, ot_ps)) for g in range(qb + 1)]
                out += [(st_evac, (h, qb, ot_ps, recb))]
                if h == HPC - 1:
                    for qt in range(qb * 4, qb * 4 + 4):
                        for nb in range(2):
                            out.append((outproj_block, (qt, nb)))
                return out

            # slot s covers nat (h,qb) of slot s plus st of slot s-1 — the
            # transposed path trails the natural path by one q-block.
            rec_sbs = []
            for s in range(NQT + 1):
                h, qb = s // NQB, s % NQB
                nat = []
                if s < NQT:
                    if qb == 0:
                        rec_sbs.append(small_p.tile(
                            [P, NQT], F32, tag="rec", name="rec_sb"))
                    nat = [(nat_stripe, (h, qt, rec_sbs[h]))
                           for qt in range(qb * 4, qb * 4 + 4)]
                st = []
                if s == 0:
                    st = [(v_proj, (sb,)) for sb in range(2)] + [(rope, (QT[1],))]
                elif s == 1:
                    st = [(v_proj, (sb,)) for sb in range(2, 4)] + [(rope, (KT[1],))]
                if s > 0:
                    hp, qbp = (s - 1) // NQB, (s - 1) % NQB
                    st = st + st_slot(hp, qbp)
                for fn, args in interleave(nat, st):
                    fn(*args)

        if DBG:
            for h in range(HPC):
                nc.sync.dma_start(DBG["ot"][h].bitcast(F32R), OT[h][:])

        if tok is not None:
            nc.sync.dma_start(tok, zeros_sb[0:1, 0:1])


_NC_CACHE = None


def _get_nc():
    global _NC_CACHE
    if _NC_CACHE is None:
        _NC_CACHE = build_bass()
    return _NC_CACHE


def _rope_tables(token_positions):
    half = DK // 2
    inv_freq = THETA ** (-np.arange(half, dtype=np.float64) * 2.0 / DK)
    ang = token_positions.astype(np.float64)[:, None] * inv_freq[None, :]  # [S, half]
    cos = np.cos(ang).astype(np.float32)   # [S, 32]
    sin = np.sin(ang).astype(np.float32)
    cosT = np.repeat(cos.T, 2, axis=0)     # [64, S]
    sinT = np.repeat(sin.T, 2, axis=0)
    sign = np.where(np.arange(DK) % 2 == 0, -1.0, 1.0).astype(np.float32)
    sinS = sinT * sign[:, None]
    cos128 = np.tile(cosT, (2, 1))         # [128, S]
    sin128 = np.tile(sinS, (2, 1))
    return np.ascontiguousarray(cos128), np.ascontiguousarray(sin128)


def make_in_maps(inputs):
    x = np.asarray(inputs["x"], dtype=np.float32)
    cos128, sin128 = _rope_tables(np.asarray(inputs["token_positions"]))
    idx = np.arange(P)
    trin_np = np.where(idx[None, :] <= idx[:, None], 0.0, NEG).astype(np.float32)
    trit_np = np.where(idx[None, :] < idx[:, None], NEG, 0.0).astype(np.float32)

    WqT = np.ascontiguousarray(np.asarray(inputs["Wq"], np.float32).T)
    WkT = np.ascontiguousarray(np.asarray(inputs["Wk"], np.float32).T)
    WvT = np.ascontiguousarray(np.asarray(inputs["Wv"], np.float32).T)
    WoT = np.ascontiguousarray(np.asarray(inputs["Wo"], np.float32).T)
    bq = np.asarray(inputs["bq"], np.float32)
    bk = np.asarray(inputs["bk"], np.float32)
    bv = np.asarray(inputs["bv"], np.float32)

    xTs = [np.ascontiguousarray(x[b].T) for b in range(B)]

    in_maps = []
    for c in range(NCORES):
        b = c // (NCORES // B)
        hg = c % (NCORES // B)
        lo, hi = hg * HPC * DK, (hg + 1) * HPC * DK
        in_maps.append({
            "xT": xTs[b],
            "wqt": np.ascontiguousarray(WqT[:, lo:hi]),
            "wkt": np.ascontiguousarray(WkT[:, lo:hi]),
            "wvt": np.ascontiguousarray(WvT[:, lo:hi]),
            "wot": np.ascontiguousarray(WoT[lo:hi, :]),
            "bqd": np.ascontiguousarray(bq[lo:hi]),
            "bkd": np.ascontiguousarray(bk[lo:hi]),
            "bvd": np.ascontiguousarray(bv[lo:hi]),
            "cosd": cos128,
            "sind": sin128,
            "trin": trin_np,
            "trit": trit_np,
        })
    return in_maps


def kernel(x, Wq, bq, Wk, bk, Wv, bv, Wo, bo, token_positions, mask):
    nc = _get_nc()
    in_maps = make_in_maps({
        "x": x, "Wq": Wq, "Wk": Wk, "Wv": Wv, "Wo": Wo,
        "bq": bq, "bk": bk, "bv": bv,
        "token_positions": token_positions,
    })

    res = run_bass_kernel_spmd(nc, in_maps, core_ids=list(range(NCORES)))
    results = res.results

    attn_full = np.empty((B, H, S, S), dtype=np.float32)
    out_full = np.empty((B, S, D), dtype=np.float32)
    bo = np.asarray(bo, np.float32)
    for b in range(B):
        acc = None
        for hg in range(NCORES // B):
            c = b * (NCORES // B) + hg
            attn_full[b, hg * HPC:(hg + 1) * HPC] = results[c]["attn"]
            acc = results[c]["outp"] if acc is None else acc + results[c]["outp"]
        out_full[b] = acc + bo[None, :]
    return out_full, attn_full
